# revision 5
# baseline (speedup 1.0000x reference)
"""Additive attention (B=64, S=2048, H=512) on 8 trn2 NeuronCores.

Strategy: data-parallel over batch (8 batches/core, no collectives).
Per batch b on each core:
  key_proj^T[h_out, s] = W1^T @ E^T     (PE; E^T via fp16 DMA-transpose loads)
  tanh fused with +query_proj bias      (ACT, per-partition bias)
  energy[1, s] = V^T @ tanh             (PE, M=1 matmuls accumulated over h)
  masked softmax over s                 (batch-on-partition [8, 2048] layout)
  attn^T via PE transpose trick         (16x [8,128]->[128,8] transposes)
  context[1, H] = sum_s attn[s]*E[s,:]  (PE, attn column as stationary weights)
All heavy matmuls in fp16 (fp32 accumulate in PSUM); softmax in fp32.
"""

import os
import sys

import numpy as np

sys.path.insert(0, "/opt/trn_rl_repo")

import concourse.bass as bass  # noqa: E402
import concourse.tile as tile  # noqa: E402
from concourse import bacc, mybir  # noqa: E402
from concourse.bass_utils import run_bass_kernel_spmd  # noqa: E402
from concourse.masks import make_identity  # noqa: E402

B, S, H = 64, 2048, 512
NCORES = 8
BPC = B // NCORES  # 8 batches per core
NEG_INF = -1e10

F16 = mybir.dt.float16
F32 = mybir.dt.float32
Tanh = mybir.ActivationFunctionType.Tanh
Exp = mybir.ActivationFunctionType.Exp
AX = mybir.AxisListType.X

_CACHE = {}
LAST_RESULT = None


def _install_ntff_hook():
    """Recreate the antenv.axon_hooks module this image lacks, so
    run_bass_kernel_spmd(trace=True) can capture NTFF profiles via the
    axon .so (same recipe as trn_agent_boot)."""
    try:
        from antenv.axon_hooks import get_axon_ntff_profile_hook  # noqa: F401

        return
    except ImportError:
        pass
    import contextlib
    import ctypes
    import types

    import antenv
    from concourse import bass_utils as _bu

    _bu.upload_artifacts = lambda tmpdir: "local"

    so_path = "/opt/axon/libaxon_pjrt.so"
    lib = ctypes.CDLL(so_path)
    if not hasattr(lib, "axon_start_nrt_profile"):
        return
    lib.axon_start_nrt_profile.argtypes = [
        ctypes.POINTER(ctypes.c_int64),
        ctypes.c_size_t,
    ]
    lib.axon_start_nrt_profile.restype = ctypes.c_int64
    lib.axon_stop_nrt_profile.argtypes = [ctypes.c_char_p]
    lib.axon_stop_nrt_profile.restype = ctypes.c_int64

    @contextlib.contextmanager
    def _hook(output_dir, device_ids):
        import jax

        jax.devices()
        if device_ids:
            ids = (ctypes.c_int64 * len(device_ids))(*device_ids)
            rc = lib.axon_start_nrt_profile(ids, len(device_ids))
        else:
            rc = lib.axon_start_nrt_profile(None, 0)
        if rc != 0:
            raise RuntimeError(f"axon_start_nrt_profile rc={rc}")
        try:
            yield
        finally:
            n = lib.axon_stop_nrt_profile(str(output_dir).encode())
            print(f"ntff profile: {n} file(s) written to {output_dir}")

    mod = types.ModuleType("antenv.axon_hooks")
    mod.set_axon_ntff_profile_hook = lambda h: None
    mod.get_axon_ntff_profile_hook = lambda: _hook
    sys.modules["antenv.axon_hooks"] = mod
    antenv.axon_hooks = mod


def _build_nc():
    nc = bacc.Bacc(
        "TRN2",
        target_bir_lowering=False,
        debug=False,
        enable_asserts=True,
        num_devices=NCORES,
    )
    e_h = nc.dram_tensor("e16", [BPC, S, H], F16, kind="ExternalInput")
    w1_h = nc.dram_tensor("w116", [H, H], F16, kind="ExternalInput")
    v_h = nc.dram_tensor("v16", [H], F16, kind="ExternalInput")
    outT_h = nc.dram_tensor("outT", [H, BPC], F32, kind="ExternalInput")
    w2_h = nc.dram_tensor("w2", [H, H], F32, kind="ExternalInput")
    mask_h = nc.dram_tensor("maskbias", [BPC, S], F32, kind="ExternalInput")
    ctx_h = nc.dram_tensor("ctx", [BPC, H], F32, kind="ExternalOutput")

    with tile.TileContext(nc) as tc:
        with (
            tc.tile_pool(name="consts", bufs=1) as consts,
            tc.tile_pool(name="small", bufs=1) as small,
            tc.tile_pool(name="et", bufs=8) as et_pool,
            tc.tile_pool(name="tanh", bufs=8) as tanh_pool,
            tc.tile_pool(name="enat", bufs=4) as enat_pool,
            tc.tile_pool(name="pkp", bufs=3, space=bass.MemorySpace.PSUM) as pkp,
            tc.tile_pool(name="psm", bufs=2, space=bass.MemorySpace.PSUM) as psm,
        ):
            # ---------------- constants ----------------
            w1_sb = consts.tile([128, 4, H], F16)  # [k_in_part, k_chunk, h_out]
            nc.sync.dma_start(
                w1_sb, w1_h.ap().rearrange("(kc kp) ho -> kp kc ho", kp=128)
            )
            v_sb = consts.tile([128, 4], F16)  # [h_part, h_chunk]
            nc.sync.dma_start(v_sb, v_h.ap().rearrange("(m p) -> p m", p=128))
            w2_sb = consts.tile([128, 4, H], F32)
            nc.sync.dma_start(
                w2_sb, w2_h.ap().rearrange("(kc kp) ho -> kp kc ho", kp=128)
            )
            outT_sb = consts.tile([128, 4, BPC], F32)
            nc.sync.dma_start(
                outT_sb, outT_h.ap().rearrange("(kc kp) b -> kp kc b", kp=128)
            )
            mask_sb = consts.tile([BPC, S], F32)
            nc.sync.dma_start(mask_sb, mask_h.ap())
            ident = consts.tile([BPC, BPC], F32)
            make_identity(nc, ident)

            # -------- query_proj^T: q2T[h_out_part, m, b] --------
            q2_ps = psm.tile([BPC, H], F32, tag="ps_small")
            for k in range(4):
                nc.tensor.matmul(
                    q2_ps,
                    outT_sb[:, k, :],
                    w2_sb[:, k, :],
                    start=(k == 0),
                    stop=(k == 3),
                )
            q2_sb = small.tile([BPC, H], F32)
            nc.vector.tensor_copy(q2_sb, q2_ps)
            q2T_ps = psm.tile([128, 4 * BPC], F32, tag="ps_small")
            for m in range(4):
                nc.tensor.transpose(
                    q2T_ps[:, m * BPC : (m + 1) * BPC],
                    q2_sb[:, m * 128 : (m + 1) * 128],
                    ident,
                )
            q2T_sb = small.tile([128, 4 * BPC], F32)
            nc.vector.tensor_copy(q2T_sb, q2T_ps)

            energy_sb = small.tile([BPC, S], F32)
            exp_sb = small.tile([BPC, S], F32)
            mx = small.tile([BPC, 1], F32)
            negmx = small.tile([BPC, 1], F32)
            sm = small.tile([BPC, 1], F32)
            rc = small.tile([BPC, 1], F32)
            ctx_sb = small.tile([BPC, H], F32)

            # -------- per batch: E^T loads, GEMM1 + tanh, energy --------
            for b in range(BPC):
                ets = []
                for k in range(4):
                    et = et_pool.tile([128, S], F16, tag="et")
                    nc.sync.dma_start(
                        et, e_h.ap()[b][:, k * 128 : (k + 1) * 128], transpose=True
                    )
                    ets.append(et)
                tanhs = []
                for m in range(4):
                    th = tanh_pool.tile([128, S], F16, tag="th")
                    for cc in range(2):
                        kp = pkp.tile([128, 1024], F32, tag="kp")
                        for h2 in range(2):
                            lo = cc * 1024 + h2 * 512
                            for k in range(4):
                                nc.tensor.matmul(
                                    kp[:, h2 * 512 : (h2 + 1) * 512],
                                    w1_sb[:, k, m * 128 : (m + 1) * 128],
                                    ets[k][:, lo : lo + 512],
                                    start=(k == 0),
                                    stop=(k == 3),
                                )
                        nc.scalar.activation(
                            th[:, cc * 1024 : (cc + 1) * 1024],
                            kp,
                            Tanh,
                            bias=q2T_sb[:, m * BPC + b : m * BPC + b + 1],
                            scale=1.0,
                        )
                    tanhs.append(th)
                estage = small.tile([1, S], F32, tag=f"estage{b % 2}")
                for c in range(4):
                    te = psm.tile([1, 512], F32, tag="ps_small")
                    for m in range(4):
                        nc.tensor.matmul(
                            te,
                            v_sb[:, m : m + 1],
                            tanhs[m][:, c * 512 : (c + 1) * 512],
                            start=(m == 0),
                            stop=(m == 3),
                        )
                    nc.vector.tensor_copy(
                        estage[:, c * 512 : (c + 1) * 512], te
                    )
                nc.sync.dma_start(energy_sb[b : b + 1, :], estage)

            # -------- masked softmax over s (batch on partitions) --------
            nc.vector.tensor_add(energy_sb, energy_sb, mask_sb)
            nc.vector.reduce_max(mx, energy_sb, axis=AX)
            nc.vector.tensor_scalar_mul(negmx, mx, -1.0)
            nc.scalar.activation(exp_sb, energy_sb, Exp, bias=negmx, scale=1.0)
            nc.vector.reduce_sum(sm, exp_sb, axis=AX)
            nc.vector.reciprocal(rc, sm)

            # -------- attn^T via PE transposes: [128, sc*8+b] --------
            at_ps = psm.tile([128, 16 * BPC], F32, tag="ps_small")
            for sc in range(16):
                nc.tensor.transpose(
                    at_ps[:, sc * BPC : (sc + 1) * BPC],
                    exp_sb[:, sc * 128 : (sc + 1) * 128],
                    ident,
                )
            attnT_sb = small.tile([128, 16 * BPC], F16)
            nc.vector.tensor_copy(attnT_sb, at_ps)

            # -------- weighted sum: context[b] = sum_s attn*E --------
            for b in range(BPC):
                en = enat_pool.tile([128, 16, H], F16, tag="en")
                nc.sync.dma_start(
                    en, e_h.ap()[b].rearrange("(sc p) h -> p sc h", p=128)
                )
                cps = psm.tile([1, H], F32, tag="ps_small")
                for sc in range(16):
                    nc.tensor.matmul(
                        cps,
                        attnT_sb[:, sc * BPC + b : sc * BPC + b + 1],
                        en[:, sc, :],
                        start=(sc == 0),
                        stop=(sc == 15),
                    )
                cstage = small.tile([1, H], F32, tag=f"cstage{b % 2}")
                nc.vector.tensor_copy(cstage, cps)
                nc.sync.dma_start(ctx_sb[b : b + 1, :], cstage)

            nc.vector.tensor_scalar_mul(ctx_sb, ctx_sb, rc)
            nc.sync.dma_start(ctx_h.ap(), ctx_sb)

    nc.compile()
    return nc


def kernel(output, encoder_outputs, encoder_sequence_lengths, W1, W2, V):
    global LAST_RESULT
    if "nc" not in _CACHE:
        _CACHE["nc"] = _build_nc()
    nc = _CACHE["nc"]

    output = np.asarray(output, dtype=np.float32)
    encoder_outputs = np.asarray(encoder_outputs, dtype=np.float32)
    seqlens = np.asarray(encoder_sequence_lengths)
    W1 = np.asarray(W1, dtype=np.float32)
    W2 = np.asarray(W2, dtype=np.float32)
    V = np.asarray(V, dtype=np.float32)

    e16 = encoder_outputs.astype(np.float16)
    w116 = np.ascontiguousarray(W1.astype(np.float16))
    v16 = np.ascontiguousarray(V[:, 0].astype(np.float16))
    mask = np.where(
        np.arange(S)[None, :] < seqlens[:, None], 0.0, NEG_INF
    ).astype(np.float32)
    outT = output[:, 0, :]  # [B, H]

    in_maps = []
    for c in range(NCORES):
        sl = slice(c * BPC, (c + 1) * BPC)
        in_maps.append(
            {
                "e16": np.ascontiguousarray(e16[sl]),
                "w116": w116,
                "v16": v16,
                "outT": np.ascontiguousarray(outT[sl].T),
                "w2": np.ascontiguousarray(W2),
                "maskbias": np.ascontiguousarray(mask[sl]),
            }
        )

    trace = os.environ.get("KERNEL_TRACE", "0") == "1"
    if trace:
        _install_ntff_hook()
    LAST_RESULT = run_bass_kernel_spmd(
        nc, in_maps, core_ids=list(range(NCORES)), trace=trace
    )
    outs = [r["ctx"] for r in LAST_RESULT.results]
    return np.concatenate(outs, axis=0).astype(np.float32)


# revision 12
# speedup vs baseline: 1.0705x; 1.0705x over previous
"""Additive attention (B=64, S=2048, H=512) on 8 trn2 NeuronCores.

Strategy: data-parallel over batch (8 batches/core, no collectives).
Per batch b on each core:
  key_proj^T[h_out, s] = W1^T @ E^T     (PE; E^T via bf16 DMA-transpose loads)
  tanh fused with +query_proj bias      (ACT, per-partition bias)
  energy[1, s] = V^T @ tanh             (PE, M=1 matmuls accumulated over h)
  masked softmax over s                 (batch-on-partition [4, 2048] groups)
  attn^T via PE transpose trick         ([4,128]->[128,4] transposes)
  context[1, H] = sum_s attn[s]*E[s,:]  (PE, attn column as stationary weights)
Heavy matmuls in bf16 (fp32 accumulate in PSUM); softmax in fp32.
Batches processed in 2 groups of 4 so group g+1's GEMMs overlap group g's
softmax + weighted-sum phase on the other engines.
"""

import os
import sys

import numpy as np

sys.path.insert(0, "/opt/trn_rl_repo")

import concourse.bass as bass  # noqa: E402
import concourse.tile as tile  # noqa: E402
from concourse import bacc, mybir  # noqa: E402
from concourse.bass_utils import run_bass_kernel_spmd  # noqa: E402
from concourse.masks import make_identity  # noqa: E402

B, S, H = 64, 2048, 512
NCORES = 8
BPC = B // NCORES  # 8 batches per core
NG = 2  # batch groups per core
GSZ = BPC // NG  # 4 batches per group
NEG_INF = -1e10

BF16 = mybir.dt.bfloat16
F32 = mybir.dt.float32
Tanh = mybir.ActivationFunctionType.Tanh
Exp = mybir.ActivationFunctionType.Exp
AX = mybir.AxisListType.X

_CACHE = {}
LAST_RESULT = None


def _install_ntff_hook():
    """Recreate the antenv.axon_hooks module this image lacks, so
    run_bass_kernel_spmd(trace=True) can capture NTFF profiles via the
    axon .so (same recipe as trn_agent_boot)."""
    try:
        from antenv.axon_hooks import get_axon_ntff_profile_hook  # noqa: F401

        return
    except ImportError:
        pass
    import contextlib
    import ctypes
    import types

    import antenv
    from concourse import bass_utils as _bu

    _bu.upload_artifacts = lambda tmpdir: "local"

    so_path = "/opt/axon/libaxon_pjrt.so"
    lib = ctypes.CDLL(so_path)
    if not hasattr(lib, "axon_start_nrt_profile"):
        return
    lib.axon_start_nrt_profile.argtypes = [
        ctypes.POINTER(ctypes.c_int64),
        ctypes.c_size_t,
    ]
    lib.axon_start_nrt_profile.restype = ctypes.c_int64
    lib.axon_stop_nrt_profile.argtypes = [ctypes.c_char_p]
    lib.axon_stop_nrt_profile.restype = ctypes.c_int64

    @contextlib.contextmanager
    def _hook(output_dir, device_ids):
        import jax

        jax.devices()
        if device_ids:
            ids = (ctypes.c_int64 * len(device_ids))(*device_ids)
            rc = lib.axon_start_nrt_profile(ids, len(device_ids))
        else:
            rc = lib.axon_start_nrt_profile(None, 0)
        if rc != 0:
            raise RuntimeError(f"axon_start_nrt_profile rc={rc}")
        try:
            yield
        finally:
            n = lib.axon_stop_nrt_profile(str(output_dir).encode())
            print(f"ntff profile: {n} file(s) written to {output_dir}")

    mod = types.ModuleType("antenv.axon_hooks")
    mod.set_axon_ntff_profile_hook = lambda h: None
    mod.get_axon_ntff_profile_hook = lambda: _hook
    sys.modules["antenv.axon_hooks"] = mod
    antenv.axon_hooks = mod


def _build_nc():
    nc = bacc.Bacc(
        "TRN2",
        target_bir_lowering=False,
        debug=False,
        enable_asserts=True,
        num_devices=NCORES,
    )
    e_h = nc.dram_tensor("e16", [BPC, S, H], BF16, kind="ExternalInput")
    w1_h = nc.dram_tensor("w116", [H, H], BF16, kind="ExternalInput")
    v_h = nc.dram_tensor("v16", [H], BF16, kind="ExternalInput")
    outT_h = nc.dram_tensor("outT", [H, BPC], F32, kind="ExternalInput")
    w2_h = nc.dram_tensor("w2", [H, H], F32, kind="ExternalInput")
    mask_h = nc.dram_tensor("maskbias", [GSZ, NG, S], F32, kind="ExternalInput")
    ctx_h = nc.dram_tensor("ctx", [BPC, H], F32, kind="ExternalOutput")

    with tile.TileContext(nc) as tc:
        with (
            tc.tile_pool(name="consts", bufs=1) as consts,
            tc.tile_pool(name="small", bufs=1) as small,
            tc.tile_pool(name="et", bufs=6) as et_pool,
            tc.tile_pool(name="tanh", bufs=6) as tanh_pool,
            tc.tile_pool(name="enat", bufs=3) as enat_pool,
            tc.tile_pool(name="pkp", bufs=3, space=bass.MemorySpace.PSUM) as pkp,
            tc.tile_pool(name="psm", bufs=2, space=bass.MemorySpace.PSUM) as psm,
        ):
            # ---------------- constants ----------------
            w1_sb = consts.tile([128, 4, H], BF16)  # [k_in_part, k_chunk, h_out]
            nc.sync.dma_start(
                w1_sb, w1_h.ap().rearrange("(kc kp) ho -> kp kc ho", kp=128)
            )
            v_sb = consts.tile([128, 4], BF16)  # [h_part, h_chunk]
            nc.sync.dma_start(v_sb, v_h.ap().rearrange("(m p) -> p m", p=128))
            w2_sb = consts.tile([128, 4, H], F32)
            nc.sync.dma_start(
                w2_sb, w2_h.ap().rearrange("(kc kp) ho -> kp kc ho", kp=128)
            )
            outT_sb = consts.tile([128, 4, BPC], F32)
            nc.sync.dma_start(
                outT_sb, outT_h.ap().rearrange("(kc kp) b -> kp kc b", kp=128)
            )
            # mask rows b = g*GSZ + p -> tile [p, g, s] so each group's rows
            # sit at partitions 0..GSZ-1
            mask_sb = consts.tile([GSZ, NG, S], F32)
            nc.sync.dma_start(mask_sb, mask_h.ap())
            ident = consts.tile([BPC, BPC], F32)
            make_identity(nc, ident)

            # -------- query_proj^T: q2T[h_out_part, m*BPC + b] --------
            q2_ps = psm.tile([BPC, H], F32, tag="ps_small")
            for k in range(4):
                nc.tensor.matmul(
                    q2_ps,
                    outT_sb[:, k, :],
                    w2_sb[:, k, :],
                    start=(k == 0),
                    stop=(k == 3),
                )
            q2_sb = small.tile([BPC, H], F32)
            nc.vector.tensor_copy(q2_sb, q2_ps)
            q2T_ps = psm.tile([128, 4 * BPC], F32, tag="ps_small")
            for m in range(4):
                nc.tensor.transpose(
                    q2T_ps[:, m * BPC : (m + 1) * BPC],
                    q2_sb[:, m * 128 : (m + 1) * 128],
                    ident,
                )
            q2T_sb = small.tile([128, 4 * BPC], F32)
            nc.vector.tensor_copy(q2T_sb, q2T_ps)

            # attnT_sb[:, sc, g, p] = attn_{b=g*GSZ+p}[sc*128 + :]
            attnT_sb = small.tile([128, 16, NG, GSZ], BF16)
            rcs = []

            for g in range(NG):
                # ---- per batch: E^T loads, GEMM1 + tanh, energy ----
                energy_g = small.tile([GSZ, S], F32, tag=f"energy{g}")
                for p in range(GSZ):
                    b = g * GSZ + p
                    ets = []
                    for k in range(4):
                        et = et_pool.tile([128, S], BF16, tag="et")
                        eng = nc.sync
                        eng.dma_start(
                            et,
                            e_h.ap()[b][:, k * 128 : (k + 1) * 128],
                            transpose=True,
                        )
                        ets.append(et)
                    tanhs = []
                    for m in range(4):
                        th = tanh_pool.tile([128, S], BF16, tag="th")
                        for cc in range(2):
                            kp = pkp.tile([128, 1024], F32, tag="kp")
                            for h2 in range(2):
                                lo = cc * 1024 + h2 * 512
                                for k in range(4):
                                    nc.tensor.matmul(
                                        kp[:, h2 * 512 : (h2 + 1) * 512],
                                        w1_sb[:, k, m * 128 : (m + 1) * 128],
                                        ets[k][:, lo : lo + 512],
                                        start=(k == 0),
                                        stop=(k == 3),
                                    )
                            nc.scalar.activation(
                                th[:, cc * 1024 : (cc + 1) * 1024],
                                kp,
                                Tanh,
                                bias=q2T_sb[:, m * BPC + b : m * BPC + b + 1],
                                scale=1.0,
                            )
                        tanhs.append(th)
                    estage = small.tile([1, S], F32, tag=f"estage{b % 2}")
                    for c in range(4):
                        te = psm.tile([1, 512], F32, tag="ps_small")
                        for m in range(4):
                            nc.tensor.matmul(
                                te,
                                v_sb[:, m : m + 1],
                                tanhs[m][:, c * 512 : (c + 1) * 512],
                                start=(m == 0),
                                stop=(m == 3),
                            )
                        nc.vector.tensor_copy(
                            estage[:, c * 512 : (c + 1) * 512], te
                        )
                    nc.sync.dma_start(energy_g[p : p + 1, :], estage)

                # ---- masked softmax over s for this group ----
                mx = small.tile([GSZ, 1], F32, tag=f"mx{g}")
                negmx = small.tile([GSZ, 1], F32, tag=f"negmx{g}")
                sm = small.tile([GSZ, 1], F32, tag=f"sm{g}")
                rc = small.tile([GSZ, 1], F32, tag=f"rc{g}")
                exp_g = small.tile([GSZ, S], F32, tag=f"exp{g}")
                nc.vector.tensor_add(energy_g, energy_g, mask_sb[:, g, :])
                nc.vector.reduce_max(mx, energy_g, axis=AX)
                nc.vector.tensor_scalar_mul(negmx, mx, -1.0)
                nc.scalar.activation(exp_g, energy_g, Exp, bias=negmx, scale=1.0)
                nc.vector.reduce_sum(sm, exp_g, axis=AX)
                nc.vector.reciprocal(rc, sm)
                rcs.append(rc)

                # ---- attn^T via PE transposes ----
                at_ps = psm.tile([128, 16 * GSZ], F32, tag="ps_small")
                for sc in range(16):
                    nc.tensor.transpose(
                        at_ps[:, sc * GSZ : (sc + 1) * GSZ],
                        exp_g[:, sc * 128 : (sc + 1) * 128],
                        ident[:GSZ, :GSZ],
                    )
                nc.vector.tensor_copy(
                    attnT_sb[:, :, g, :],
                    at_ps.rearrange("p (sc q) -> p sc q", q=GSZ),
                )

                # ---- weighted sum: context[b] = sum_s attn*E ----
                ctx_g = small.tile([GSZ, H], F32, tag=f"ctx{g}")
                for p in range(GSZ):
                    b = g * GSZ + p
                    en = enat_pool.tile([128, 16, H], BF16, tag="en")
                    nc.sync.dma_start(
                        en, e_h.ap()[b].rearrange("(sc p) h -> p sc h", p=128)
                    )
                    cps = psm.tile([1, H], F32, tag="ps_small")
                    for sc in range(16):
                        nc.tensor.matmul(
                            cps,
                            attnT_sb[:, sc, g, p : p + 1],
                            en[:, sc, :],
                            start=(sc == 0),
                            stop=(sc == 15),
                        )
                    cstage = small.tile([1, H], F32, tag=f"cstage{b % 2}")
                    nc.vector.tensor_copy(cstage, cps)
                    nc.sync.dma_start(ctx_g[p : p + 1, :], cstage)

                nc.vector.tensor_scalar_mul(ctx_g, ctx_g, rcs[g])
                nc.sync.dma_start(ctx_h.ap()[g * GSZ : (g + 1) * GSZ, :], ctx_g)

    nc.compile()
    return nc


def kernel(output, encoder_outputs, encoder_sequence_lengths, W1, W2, V):
    global LAST_RESULT
    if "nc" not in _CACHE:
        _CACHE["nc"] = _build_nc()
    nc = _CACHE["nc"]

    import ml_dtypes

    bf16 = ml_dtypes.bfloat16

    output = np.asarray(output, dtype=np.float32)
    encoder_outputs = np.asarray(encoder_outputs, dtype=np.float32)
    seqlens = np.asarray(encoder_sequence_lengths)
    W1 = np.asarray(W1, dtype=np.float32)
    W2 = np.asarray(W2, dtype=np.float32)
    V = np.asarray(V, dtype=np.float32)

    e16 = encoder_outputs.astype(bf16)
    w116 = np.ascontiguousarray(W1.astype(bf16))
    v16 = np.ascontiguousarray(V[:, 0].astype(bf16))
    mask = np.where(
        np.arange(S)[None, :] < seqlens[:, None], 0.0, NEG_INF
    ).astype(np.float32)
    outT = output[:, 0, :]  # [B, H]

    in_maps = []
    for c in range(NCORES):
        sl = slice(c * BPC, (c + 1) * BPC)
        in_maps.append(
            {
                "e16": np.ascontiguousarray(e16[sl]),
                "w116": w116,
                "v16": v16,
                "outT": np.ascontiguousarray(outT[sl].T),
                "w2": np.ascontiguousarray(W2),
                "maskbias": np.ascontiguousarray(mask[sl].reshape(2, 4, S).transpose(1, 0, 2)),
            }
        )

    trace = os.environ.get("KERNEL_TRACE", "0") == "1"
    if trace:
        _install_ntff_hook()
    LAST_RESULT = run_bass_kernel_spmd(
        nc, in_maps, core_ids=list(range(NCORES)), trace=trace
    )
    outs = [r["ctx"] for r in LAST_RESULT.results]
    return np.concatenate(outs, axis=0).astype(np.float32)
